# revision 10
# baseline (speedup 1.0000x reference)
"""Trainium2 Bass kernel for a cross+self attention decoder block.

Reference computation (per batch b):
  xn = LN_seq(x); x += MHA(xn, z)      (cross attention)
  xn = LN_seq(x); x += MHA(xn, xn)     (self attention)
  xn = LN_seq(x); x += GELU(xn@W1+b1)@W2+b2
where LN_seq normalizes over the SEQUENCE axis (dim=-2), unbiased std,
eps added to sigma; attention scale is 1/sqrt(D).

Sharding: 8 cores = 4 batches x 2 sequence halves (784 rows each).
Each core computes its own 784-row slice of the output. Cross-attention
K/V come from the full z (a kernel input, so no communication). The
self-attention K/V need the full post-cross residual stream, exchanged
with the pair peer via one pairwise AllGather (bf16). LN1 stats are
computed locally (both halves of x are inputs); LN2 stats from the
gathered x2; LN3 stats via a tiny pairwise AllReduce of partial sums.

On-chip layout: activations are stored transposed (feature on the
partition axis, sequence on the free axis) which serves both matmuls
(contraction on partitions) and the sequence-axis layernorm (free-axis
reduction). The host pre-packs every DRAM tensor into the exact SBUF
tile layout so all DMAs are identity-shaped.
"""

import math

import numpy as np
import ml_dtypes

BF16 = ml_dtypes.bfloat16

# Problem constants (hardcoded per the harness contract).
D = 512
HEADS = 8
HD = 64  # head dim
HID = 2048
B = 4
S = 1568
SC = 784  # per-core sequence chunk
EPS = 1e-6
NCORES = 8

P = 128  # partitions
DT = D // P  # 4 feature tiles
QB = 2  # q blocks per chunk
QW = SC // QB  # 392
NKC = 13  # key chunks: 12 x 128 + 32
KSZ = [P] * 12 + [32]
HT = HID // P  # 16 hidden tiles
SCALE = 1.0 / math.sqrt(D)
VAR_CORR = S / (S - 1.0)  # unbiased variance correction

# consts column map (all fp32, shape (128, NC)); each 512-vector -> 4 cols
C_BQ_C, C_BK_C, C_BO_C = 0, 4, 8
C_BQ_S, C_BK_S, C_BO_S = 12, 16, 20
C_G1, C_B1, C_G2, C_B2, C_G3, C_B3 = 24, 28, 32, 36, 40, 44
C_MB2 = 48  # mlp output bias b2
C_MB1 = 52  # mlp hidden bias b1: 16 cols
NC = 68

_COMPILED = None  # (nc, names) cache


def _pack_pd(v):
    """(512,) -> (128, 4): column c holds v[c*128:(c+1)*128]."""
    return np.ascontiguousarray(v.reshape(DT, P).T.astype(np.float32))


def _pack_w(w):
    """(din, dout) -> (128, din//128, dout) bf16 stationary layout."""
    din, dout = w.shape
    return np.ascontiguousarray(
        w.reshape(din // P, P, dout).transpose(1, 0, 2).astype(BF16)
    )


def _pack_xT(x, dtype):
    """(rows, 512) -> (128, 4, rows): [p, c, s] = x[s, c*128+p]."""
    rows = x.shape[0]
    return np.ascontiguousarray(
        x.T.reshape(DT, P, rows).transpose(1, 0, 2).astype(dtype)
    )


def _build():
    import concourse.bass as bass
    import concourse.mybir as mybir
    import concourse.tile as tile
    from concourse import bacc

    f32 = mybir.dt.float32
    bf16 = mybir.dt.bfloat16
    AF = mybir.ActivationFunctionType
    OP = mybir.AluOpType

    nc = bacc.Bacc("TRN2", target_bir_lowering=False, debug=False, num_devices=NCORES)

    def din(name, shape, dt=f32):
        return nc.dram_tensor(name, list(shape), dt, kind="ExternalInput").ap()

    x_own = din("x_own", (P, DT, SC))
    x_peer = din("x_peer", (P, DT, SC), bf16)
    zT = din("zT", (P, DT, S), bf16)
    w_attn = {}
    for pre in ("c", "s"):
        for nm in ("wq", "wk", "wv", "wo"):
            w_attn[f"{nm}_{pre}"] = din(f"{nm}_{pre}", (P, DT, D), bf16)
    w1 = din("w1", (P, DT, HID), bf16)
    w2 = din("w2", (P, HT, D), bf16)
    consts = din("consts", (P, NC))
    outp = nc.dram_tensor("outp", [P, DT, SC], f32, kind="ExternalOutput").ap()

    PAIRS = [[0, 1], [2, 3], [4, 5], [6, 7]]

    from contextlib import ExitStack

    with tile.TileContext(nc) as tc, ExitStack() as stack:
        wpool = stack.enter_context(tc.tile_pool(name="wpool", bufs=1))
        iop = stack.enter_context(tc.tile_pool(name="iop", bufs=1))
        fullp = stack.enter_context(tc.tile_pool(name="fullp", bufs=1))
        xp = stack.enter_context(tc.tile_pool(name="xp", bufs=1))
        ap_pool = stack.enter_context(tc.tile_pool(name="ap_pool", bufs=3))
        stp = stack.enter_context(tc.tile_pool(name="stp", bufs=2))
        dramp = stack.enter_context(tc.tile_pool(name="dramp", bufs=1, space="DRAM"))

        # ---- load inputs + phase-1 weights first, bulk weights after ----
        xo_sb = iop.tile([P, DT, SC], f32, name="xo_sb", tag="resid", bufs=2)
        nc.sync.dma_start(out=xo_sb[:], in_=x_own[:])
        xp_sb = iop.tile([P, DT, SC], bf16, name="xp_sb", tag="xpeer")
        nc.sync.dma_start(out=xp_sb[:], in_=x_peer[:])
        z_sb = fullp.tile([P, DT, S], bf16, name="z_sb", tag="fullA", bufs=2)
        nc.sync.dma_start(out=z_sb[:], in_=zT[:])
        cst = wpool.tile([P, NC], f32, name="cst", tag="cst")
        nc.sync.dma_start(out=cst[:], in_=consts[:])
        wsb = {}
        for nm in ("wv_c", "wk_c", "wq_c", "wo_c", "wq_s", "wk_s", "wv_s", "wo_s"):
            t = wpool.tile([P, DT, D], bf16, name=f"sb_{nm}", tag=f"sb_{nm}")
            nc.sync.dma_start(out=t[:], in_=w_attn[nm][:])
            wsb[nm] = t
        w1_sb = wpool.tile([P, DT, HID], bf16, name="w1_sb", tag="w1_sb")
        nc.sync.dma_start(out=w1_sb[:], in_=w1[:])
        w2_sb = wpool.tile([P, HT, D], bf16, name="w2_sb", tag="w2_sb")
        nc.sync.dma_start(out=w2_sb[:], in_=w2[:])

        # ---- helpers ----
        def ln_coeffs(mv, g_col, b_col, name):
            """mv: (P, DT, 2) [mean, biased var over full seq].
            Returns (scale, bias) tiles (P, DT) for xn = x*scale + bias."""
            mu = stp.tile([P, DT], f32, name=f"mu_{name}", tag="ln_mu")
            nc.vector.tensor_copy(out=mu[:], in_=mv[:, :, 0])
            sig = stp.tile([P, DT], f32, name=f"sig_{name}", tag="ln_sig")
            # sigma = sqrt(var * N/(N-1)) + eps
            nc.scalar.activation(
                out=sig[:], in_=mv[:, :, 1], func=AF.Sqrt, scale=float(VAR_CORR)
            )
            nc.vector.tensor_scalar_add(sig[:], sig[:], float(EPS))
            inv = stp.tile([P, DT], f32, name=f"inv_{name}", tag="ln_inv")
            nc.vector.reciprocal(out=inv[:], in_=sig[:])
            sc = stp.tile([P, DT], f32, name=f"sc_{name}", tag="ln_sc")
            nc.vector.tensor_tensor(
                out=sc[:], in0=cst[:, g_col : g_col + DT], in1=inv[:], op=OP.mult
            )
            bi = stp.tile([P, DT], f32, name=f"bi_{name}", tag="ln_bi")
            nc.vector.tensor_tensor(out=bi[:], in0=mu[:], in1=sc[:], op=OP.mult)
            nc.vector.tensor_tensor(
                out=bi[:], in0=cst[:, b_col : b_col + DT], in1=bi[:], op=OP.subtract
            )
            return sc, bi

        def proj_kT(ppool, w_t, src_full, bias_col, dst, name):
            """K^T-style projection over the full sequence.
            src_full: (P, DT, S) bf16; dst: (P, DT, S) bf16 = W.T@src + b."""
            for m in range(DT):
                for half in range(2):
                    ps = ppool.tile(
                        [P, QB, 512], f32, name=f"ps_{name}{m}{half}", tag="projps"
                    )
                    for kb in range(QB):
                        s0 = (half * QB + kb) * QW
                        for c in range(DT):
                            nc.tensor.matmul(
                                ps[:, kb, 0:QW],
                                w_t[:, c, m * P : (m + 1) * P],
                                src_full[:, c, s0 : s0 + QW],
                                start=(c == 0),
                                stop=(c == DT - 1),
                            )
                    nc.vector.tensor_scalar(
                        out=dst[:, m, half * SC : (half + 1) * SC],
                        in0=ps[:, :, 0:QW],
                        scalar1=cst[:, bias_col + m : bias_col + m + 1],
                        scalar2=None,
                        op0=OP.add,
                    )

        def proj_v(ppool, w_t, src_full, vdst, name):
            """V in natural layout with an appended ones column.
            vdst: (P, NKC, HEADS, HD+1) bf16."""
            for j in range(NKC):
                kj = KSZ[j]
                k0 = j * P
                ps = ppool.tile([P, 512], f32, name=f"psv_{name}{j}", tag="vps")
                for c in range(DT):
                    nc.tensor.matmul(
                        ps[0:kj, :],
                        src_full[:, c, k0 : k0 + kj],
                        w_t[:, c, :],
                        start=(c == 0),
                        stop=(c == DT - 1),
                    )
                nc.vector.tensor_copy(
                    out=vdst[0:kj, j, :, 0:HD],
                    in_=ps[0:kj, :].rearrange("p (h d) -> p h d", h=HEADS),
                )

        def proj_q(ppool, w_t, xn_own, bias_col, qdst, name):
            """Q^T projection from own chunk. qdst: (P, DT, QB, QW) bf16."""
            for m in range(DT):
                ps = ppool.tile([P, QB, 512], f32, name=f"psq_{name}{m}", tag="projps")
                for qb in range(QB):
                    for c in range(DT):
                        nc.tensor.matmul(
                            ps[:, qb, 0:QW],
                            w_t[:, c, m * P : (m + 1) * P],
                            xn_own[:, c, qb, :],
                            start=(c == 0),
                            stop=(c == DT - 1),
                        )
                nc.vector.tensor_scalar(
                    out=qdst[:, m, :, :],
                    in0=ps[:, :, 0:QW],
                    scalar1=cst[:, bias_col + m : bias_col + m + 1],
                    scalar2=None,
                    op0=OP.add,
                )

        def attention(kT, vfull, qT, ysb, name):
            """Per-head attention. kT: (P, DT, S) bf16, vfull: (P, NKC, HEADS, HD+1),
            qT: (P, DT, QB, QW). Writes ysb (P, DT, QB, QW) bf16 = normalized out."""
            with tc.tile_pool(name=f"att_{name}", bufs=1, space="PSUM") as attps:
                for h in range(HEADS):
                    off = (h % 2) * HD
                    t = h // 2
                    yps = attps.tile(
                        [HD + 1, QB, 512], f32, name=f"yps_{name}{h}", tag="yps", bufs=2
                    )
                    for j in range(NKC):
                        kj = KSZ[j]
                        k0 = j * P
                        sps = attps.tile(
                            [P, QB, 512], f32, name=f"sps_{name}{h}{j}", tag="sps",
                            bufs=2,
                        )
                        for qb in range(QB):
                            nc.tensor.matmul(
                                sps[0:kj, qb, 0:QW],
                                kT[off : off + HD, t, k0 : k0 + kj],
                                qT[off : off + HD, t, qb, :],
                                start=True,
                                stop=True,
                            )
                        a_sb = ap_pool.tile(
                            [P, QB, QW], bf16, name=f"a_{name}{h}{j}", tag="a_sb",
                            bufs=3,
                        )
                        nc.scalar.activation(
                            out=a_sb[0:kj, :, :],
                            in_=sps[0:kj, :, 0:QW],
                            func=AF.Exp,
                            scale=float(SCALE),
                        )
                        for qb in range(QB):
                            nc.tensor.matmul(
                                yps[:, qb, 0:QW],
                                vfull[0:kj, j, h, :],
                                a_sb[0:kj, qb, :],
                                start=(j == 0),
                                stop=(j == NKC - 1),
                            )
                    # normalize: y / l  (l accumulated in partition HD)
                    rl = ap_pool.tile([1, QB, QW], f32, name=f"rl_{name}{h}", tag="rl", bufs=2)
                    nc.vector.reciprocal(
                        out=rl[:], in_=yps[HD : HD + 1, :, 0:QW]
                    )
                    rlb = ap_pool.tile(
                        [HD, QB, QW], f32, name=f"rlb_{name}{h}", tag="rlb", bufs=2
                    )
                    nc.gpsimd.partition_broadcast(rlb[:], rl[:])
                    nc.vector.tensor_tensor(
                        out=ysb[off : off + HD, t, :, :],
                        in0=yps[0:HD, :, 0:QW],
                        in1=rlb[:],
                        op=OP.mult,
                    )

        def out_proj(ppool, w_t, ysb, bias_col, resid_in, resid_out, name):
            """resid_out = resid_in + W.T@y + bias (per-partition)."""
            for m in range(DT):
                ps = ppool.tile([P, QB, 512], f32, name=f"pso_{name}{m}", tag="projps")
                for qb in range(QB):
                    for c in range(DT):
                        nc.tensor.matmul(
                            ps[:, qb, 0:QW],
                            w_t[:, c, m * P : (m + 1) * P],
                            ysb[:, c, qb, :],
                            start=(c == 0),
                            stop=(c == DT - 1),
                        )
                nc.vector.scalar_tensor_tensor(
                    out=resid_out[:, m, :].rearrange("p (a b) -> p a b", a=QB),
                    in0=ps[:, :, 0:QW],
                    scalar=cst[:, bias_col + m : bias_col + m + 1],
                    in1=resid_in[:, m, :].rearrange("p (a b) -> p a b", a=QB),
                    op0=OP.add,
                    op1=OP.add,
                )

        # ================= LN1 (stats over own + peer x) =================
        mv1 = stp.tile([P, DT, 2], f32, name="mv1", tag="mv")
        for c in range(DT):
            st1 = stp.tile([P, 4, 6], f32, name=f"st1_{c}", tag="bnst")
            for half in range(QB):
                nc.vector.bn_stats(
                    out=st1[:, half, :], in_=xo_sb[:, c, half * QW : (half + 1) * QW]
                )
                nc.vector.bn_stats(
                    out=st1[:, 2 + half, :],
                    in_=xp_sb[:, c, half * QW : (half + 1) * QW],
                )
            nc.vector.bn_aggr(out=mv1[:, c, :], in_=st1[:])
        sc1, bi1 = ln_coeffs(mv1, C_G1, C_B1, "ln1")
        xn1 = xp.tile([P, DT, QB, QW], bf16, name="xn1", tag="xn", bufs=2)
        for c in range(DT):
            nc.vector.tensor_scalar(
                out=xn1[:, c, :, :],
                in0=xo_sb[:, c, :].rearrange("p (a b) -> p a b", a=QB),
                scalar1=sc1[:, c : c + 1],
                scalar2=bi1[:, c : c + 1],
                op0=OP.mult,
                op1=OP.add,
            )

        # ================= cross attention =================
        kT_c = fullp.tile([P, DT, S], bf16, name="kT_c", tag="kt")
        v_c = fullp.tile([P, NKC, HEADS, HD + 1], bf16, name="v_c", tag="vf")
        nc.vector.memset(v_c[:, :, :, HD : HD + 1], 1.0)
        with tc.tile_pool(name="projps_c", bufs=2, space="PSUM") as ppool:
            proj_v(ppool, wsb["wv_c"], z_sb, v_c, "vc")
            proj_kT(ppool, wsb["wk_c"], z_sb, C_BK_C, kT_c, "kc")
            qT_c = xp.tile([P, DT, QB, QW], bf16, name="qT_c", tag="qt", bufs=2)
            proj_q(ppool, wsb["wq_c"], xn1, C_BQ_C, qT_c, "qc")
        y_c = xp.tile([P, DT, QB, QW], bf16, name="y_c", tag="ysb", bufs=2)
        attention(kT_c, v_c, qT_c, y_c, "c")
        x2 = iop.tile([P, DT, SC], f32, name="x2", tag="resid", bufs=2)
        with tc.tile_pool(name="projps_o1", bufs=2, space="PSUM") as ppool:
            out_proj(ppool, wsb["wo_c"], y_c, C_BO_C, xo_sb, x2, "oc")

        # ========== LN2 partial stats -> pairwise AllReduce (issued FIRST
        # on the collective queue so the big x2 AllGather hides behind the
        # stats-dependent Q/xn2 work), then the x2 exchange ==========
        mv2o = stp.tile([P, DT, 2], f32, name="mv2o", tag="mv")
        for c in range(DT):
            st2 = stp.tile([P, 2, 6], f32, name=f"st2_{c}", tag="bnst")
            for half in range(QB):
                nc.vector.bn_stats(
                    out=st2[:, half, :], in_=x2[:, c, half * QW : (half + 1) * QW]
                )
            nc.vector.bn_aggr(out=mv2o[:, c, :], in_=st2[:])
        st2_sb = stp.tile([P, 2, DT], f32, name="st2_sb", tag="stats")
        nc.vector.tensor_copy(out=st2_sb[:, 0, :], in_=mv2o[:, :, 0])
        nc.vector.tensor_tensor(
            out=st2_sb[:, 1, :], in0=mv2o[:, :, 0], in1=mv2o[:, :, 0], op=OP.mult
        )
        nc.vector.tensor_tensor(
            out=st2_sb[:, 1, :], in0=st2_sb[:, 1, :], in1=mv2o[:, :, 1], op=OP.add
        )
        st2_in = dramp.tile([P, 2, DT], f32, name="st2_in", tag="st2in")
        nc.gpsimd.dma_start(out=st2_in[:], in_=st2_sb[:])
        st2_out = dramp.tile([P, 2, DT], f32, name="st2_out", tag="st2out")
        nc.gpsimd.collective_compute(
            "AllReduce",
            mybir.AluOpType.add,
            replica_groups=PAIRS,
            ins=[st2_in[:].opt()],
            outs=[st2_out[:].opt()],
        )

        x2_bf = xp.tile([P, DT, SC], bf16, name="x2_bf", tag="xn", bufs=2)
        nc.vector.tensor_copy(out=x2_bf[:], in_=x2[:])
        bounce_in = dramp.tile([P, DT, SC], bf16, name="bounce_in", tag="bin")
        nc.gpsimd.dma_start(out=bounce_in[:], in_=x2_bf[:])
        bounce_out = dramp.tile([2, P, DT, SC], bf16, name="bounce_out", tag="bout")
        nc.gpsimd.collective_compute(
            "AllGather",
            mybir.AluOpType.bypass,
            replica_groups=PAIRS,
            ins=[bounce_in[:].opt()],
            outs=[bounce_out[:].opt()],
        )
        x2f = fullp.tile([P, DT, S], bf16, name="x2f", tag="fullA", bufs=2)
        for s in range(2):
            nc.gpsimd.dma_start(
                out=x2f[:, :, s * SC : (s + 1) * SC], in_=bounce_out[s]
            )

        # ================= LN2 coefficients from the reduced stats ========
        st2_rd = stp.tile([P, 2, DT], f32, name="st2_rd", tag="stats")
        nc.gpsimd.dma_start(out=st2_rd[:], in_=st2_out[:])
        mv2 = stp.tile([P, DT, 2], f32, name="mv2", tag="mv")
        nc.vector.tensor_scalar_mul(mv2[:, :, 0], st2_rd[:, 0, :], 0.5)
        tmp2 = stp.tile([P, DT], f32, name="tmp2", tag="tmp3")
        nc.vector.tensor_tensor(
            out=tmp2[:], in0=mv2[:, :, 0], in1=mv2[:, :, 0], op=OP.mult
        )
        nc.vector.tensor_scalar(
            out=mv2[:, :, 1],
            in0=st2_rd[:, 1, :],
            scalar1=0.5,
            scalar2=None,
            op0=OP.mult,
        )
        nc.vector.tensor_tensor(
            out=mv2[:, :, 1], in0=mv2[:, :, 1], in1=tmp2[:], op=OP.subtract
        )
        sc2, bi2 = ln_coeffs(mv2, C_G2, C_B2, "ln2")
        xn2o = xp.tile([P, DT, QB, QW], bf16, name="xn2o", tag="xn", bufs=2)
        for c in range(DT):
            nc.vector.tensor_scalar(
                out=xn2o[:, c, :, :],
                in0=x2[:, c, :].rearrange("p (a b) -> p a b", a=QB),
                scalar1=sc2[:, c : c + 1],
                scalar2=bi2[:, c : c + 1],
                op0=OP.mult,
                op1=OP.add,
            )
        kT_s = fullp.tile([P, DT, S], bf16, name="kT_s", tag="kt")
        v_s = fullp.tile([P, NKC, HEADS, HD + 1], bf16, name="v_s", tag="vf")
        nc.vector.memset(v_s[:, :, :, HD : HD + 1], 1.0)
        with tc.tile_pool(name="projps_s", bufs=2, space="PSUM") as ppool:
            qT_s = xp.tile([P, DT, QB, QW], bf16, name="qT_s", tag="qt", bufs=2)
            proj_q(ppool, wsb["wq_s"], xn2o, C_BQ_S, qT_s, "qs")
            xn2f = fullp.tile([P, DT, S], bf16, name="xn2f", tag="fullA", bufs=2)
            for c in range(DT):
                nc.vector.tensor_scalar(
                    out=xn2f[:, c, :].rearrange("p (a b) -> p a b", a=4),
                    in0=x2f[:, c, :].rearrange("p (a b) -> p a b", a=4),
                    scalar1=sc2[:, c : c + 1],
                    scalar2=bi2[:, c : c + 1],
                    op0=OP.mult,
                    op1=OP.add,
                )
            proj_v(ppool, wsb["wv_s"], xn2f, v_s, "vs")
            proj_kT(ppool, wsb["wk_s"], xn2f, C_BK_S, kT_s, "ks")
        y_s = xp.tile([P, DT, QB, QW], bf16, name="y_s", tag="ysb", bufs=2)
        attention(kT_s, v_s, qT_s, y_s, "s")
        x3 = iop.tile([P, DT, SC], f32, name="x3", tag="resid", bufs=2)
        with tc.tile_pool(name="projps_o2", bufs=2, space="PSUM") as ppool:
            out_proj(ppool, wsb["wo_s"], y_s, C_BO_S, x2, x3, "os")

        # ================= LN3 (pairwise stats AllReduce) =================
        mv3 = stp.tile([P, DT, 2], f32, name="mv3", tag="mv")
        for c in range(DT):
            st3 = stp.tile([P, 2, 6], f32, name=f"st3_{c}", tag="bnst")
            for half in range(QB):
                nc.vector.bn_stats(
                    out=st3[:, half, :], in_=x3[:, c, half * QW : (half + 1) * QW]
                )
            nc.vector.bn_aggr(out=mv3[:, c, :], in_=st3[:])
        # partial sums: [mean_own, E[x^2]_own] packed as (P, 2, DT)
        stats_sb = stp.tile([P, 2, DT], f32, name="stats_sb", tag="stats")
        nc.vector.tensor_copy(out=stats_sb[:, 0, :], in_=mv3[:, :, 0])
        nc.vector.tensor_tensor(
            out=stats_sb[:, 1, :], in0=mv3[:, :, 0], in1=mv3[:, :, 0], op=OP.mult
        )
        nc.vector.tensor_tensor(
            out=stats_sb[:, 1, :], in0=stats_sb[:, 1, :], in1=mv3[:, :, 1], op=OP.add
        )
        st_in = dramp.tile([P, 2, DT], f32, name="st_in", tag="stin")
        nc.gpsimd.dma_start(out=st_in[:], in_=stats_sb[:])
        st_out = dramp.tile([P, 2, DT], f32, name="st_out", tag="stout")
        nc.gpsimd.collective_compute(
            "AllReduce",
            mybir.AluOpType.add,
            replica_groups=PAIRS,
            ins=[st_in[:].opt()],
            outs=[st_out[:].opt()],
        )
        stats_rd = stp.tile([P, 2, DT], f32, name="stats_rd", tag="stats")
        nc.gpsimd.dma_start(out=stats_rd[:], in_=st_out[:])
        mv3g = stp.tile([P, DT, 2], f32, name="mv3g", tag="mv")
        # mean = sum/2 ; var = sum(E[x^2])/2 - mean^2
        nc.vector.tensor_scalar_mul(mv3g[:, :, 0], stats_rd[:, 0, :], 0.5)
        tmp3 = stp.tile([P, DT], f32, name="tmp3", tag="tmp3")
        nc.vector.tensor_tensor(
            out=tmp3[:], in0=mv3g[:, :, 0], in1=mv3g[:, :, 0], op=OP.mult
        )
        nc.vector.tensor_scalar(
            out=mv3g[:, :, 1],
            in0=stats_rd[:, 1, :],
            scalar1=0.5,
            scalar2=None,
            op0=OP.mult,
        )
        nc.vector.tensor_tensor(
            out=mv3g[:, :, 1], in0=mv3g[:, :, 1], in1=tmp3[:], op=OP.subtract
        )
        sc3, bi3 = ln_coeffs(mv3g, C_G3, C_B3, "ln3")
        xn3 = xp.tile([P, DT, QB, QW], bf16, name="xn3", tag="xn", bufs=2)
        for c in range(DT):
            nc.vector.tensor_scalar(
                out=xn3[:, c, :, :],
                in0=x3[:, c, :].rearrange("p (a b) -> p a b", a=QB),
                scalar1=sc3[:, c : c + 1],
                scalar2=bi3[:, c : c + 1],
                op0=OP.mult,
                op1=OP.add,
            )

        # ================= MLP =================
        out_sb = iop.tile([P, DT, SC], f32, name="out_sb", tag="resid", bufs=2)
        with tc.tile_pool(name="mlpps", bufs=1, space="PSUM") as mpool:
            for qb in range(QB):
                ops = []
                for m in range(DT):
                    t = mpool.tile(
                        [P, 512], f32, name=f"mo_{qb}{m}", tag=f"mo{m}", bufs=1
                    )
                    ops.append(t)
                for ht in range(HT):
                    hps = mpool.tile([P, 512], f32, name=f"hps_{qb}{ht}", tag="hps",
                                     bufs=2)
                    for c in range(DT):
                        nc.tensor.matmul(
                            hps[:, 0:QW],
                            w1_sb[:, c, ht * P : (ht + 1) * P],
                            xn3[:, c, qb, :],
                            start=(c == 0),
                            stop=(c == DT - 1),
                        )
                    h_sb = ap_pool.tile([P, QW], bf16, name=f"h_{qb}{ht}", tag="h_sb", bufs=3)
                    nc.scalar.activation(
                        out=h_sb[:],
                        in_=hps[:, 0:QW],
                        func=AF.Gelu,
                        bias=cst[:, C_MB1 + ht : C_MB1 + ht + 1],
                    )
                    for m in range(DT):
                        nc.tensor.matmul(
                            ops[m][:, 0:QW],
                            w2_sb[:, ht, m * P : (m + 1) * P],
                            h_sb[:],
                            start=(ht == 0),
                            stop=(ht == HT - 1),
                        )
                for m in range(DT):
                    nc.vector.scalar_tensor_tensor(
                        out=out_sb[:, m, qb * QW : (qb + 1) * QW],
                        in0=ops[m][:, 0:QW],
                        scalar=cst[:, C_MB2 + m : C_MB2 + m + 1],
                        in1=x3[:, m, qb * QW : (qb + 1) * QW],
                        op0=OP.add,
                        op1=OP.add,
                    )
        nc.sync.dma_start(out=outp[:], in_=out_sb[:])

    nc.compile()
    return nc


def _get_compiled():
    global _COMPILED
    if _COMPILED is None:
        _COMPILED = _build()
    return _COMPILED


def _prepare_in_maps(x, z, params):
    x = np.asarray(x, np.float32)
    z = np.asarray(z, np.float32)

    def npf(a):
        return np.asarray(a, np.float32)

    shared = {}
    for pre, key in (("c", "cross"), ("s", "self")):
        p = params[key]
        shared[f"wq_{pre}"] = _pack_w(npf(p["Wq"]))
        shared[f"wk_{pre}"] = _pack_w(npf(p["Wk"]))
        shared[f"wv_{pre}"] = _pack_w(npf(p["Wv"]))
        shared[f"wo_{pre}"] = _pack_w(npf(p["Wo"]))
    shared["w1"] = _pack_w(npf(params["mlp"]["W1"]))
    shared["w2"] = _pack_w(npf(params["mlp"]["W2"]))

    cst = np.zeros((P, NC), np.float32)
    for pre, key, off in (("c", "cross", 0), ("s", "self", 12)):
        p = params[key]
        cst[:, off + 0 : off + 4] = _pack_pd(npf(p["bq"]))
        cst[:, off + 4 : off + 8] = _pack_pd(npf(p["bk"]))
        bo2 = npf(p["bv"]) @ npf(p["Wo"]) + npf(p["bo"])
        cst[:, off + 8 : off + 12] = _pack_pd(bo2)
    cst[:, C_G1 : C_G1 + 4] = _pack_pd(npf(params["norm1"]["g"]))
    cst[:, C_B1 : C_B1 + 4] = _pack_pd(npf(params["norm1"]["b"]))
    cst[:, C_G2 : C_G2 + 4] = _pack_pd(npf(params["norm2"]["g"]))
    cst[:, C_B2 : C_B2 + 4] = _pack_pd(npf(params["norm2"]["b"]))
    cst[:, C_G3 : C_G3 + 4] = _pack_pd(npf(params["norm3"]["g"]))
    cst[:, C_B3 : C_B3 + 4] = _pack_pd(npf(params["norm3"]["b"]))
    cst[:, C_MB2 : C_MB2 + 4] = _pack_pd(npf(params["mlp"]["b2"]))
    cst[:, C_MB1 : C_MB1 + HT] = np.ascontiguousarray(
        npf(params["mlp"]["b1"]).reshape(HT, P).T
    )
    shared["consts"] = cst

    in_maps = []
    for core in range(NCORES):
        b, half = core // 2, core % 2
        rows = slice(half * SC, (half + 1) * SC)
        peer_rows = slice((1 - half) * SC, (2 - half) * SC)
        m = dict(shared)
        m["x_own"] = _pack_xT(x[b, rows], np.float32)
        m["x_peer"] = _pack_xT(x[b, peer_rows], BF16)
        m["zT"] = _pack_xT(z[b], BF16)
        in_maps.append(m)
    return in_maps


def _assemble(results):
    out = np.empty((B, S, D), np.float32)
    for core in range(NCORES):
        b, half = core // 2, core % 2
        chunk = results[core]["outp"]  # (P, DT, SC)
        out[b, half * SC : (half + 1) * SC, :] = (
            chunk.transpose(1, 0, 2).reshape(D, SC).T
        )
    return out


def run_on_hw(x, z, params, trace=False):
    from concourse import bass_utils

    nc = _get_compiled()
    in_maps = _prepare_in_maps(x, z, params)
    res = bass_utils.run_bass_kernel_spmd(
        nc, in_maps, core_ids=list(range(NCORES)), trace=trace
    )
    return _assemble(res.results), res


def kernel(x, z, params):
    out, _ = run_on_hw(x, z, params, trace=False)
    return out
